# revision 1
# baseline (speedup 1.0000x reference)
"""Trainium2 Bass kernel for nn_DomainAdaptation (sparse feature-attention + dual MLP).

Math (reference):
    S = Q^T K                        [D, D], contraction over N
    L = exp(S - S*I/sqrt(D))
    scores = softmax(L, axis=-1)
    attn = (scores @ V^T)^T          [N, D]
    dom_q = relu(attn @ Wq1 + bq1) @ Wq2 + bq2
    dom_k = relu(attn @ Wk1 + bk1) @ Wk2 + bk2

Key restructuring: attn = V @ scores^T, so
    attn @ W1 = V @ (scores^T @ W1) = V @ M1
and attn is never materialized. Per core (N sharded 8 ways):
    phase 1: S_partial = Qc^T Kc  (bf16 matmuls, f32 accum)
    ReduceScatter(S) -> each core owns a 128-row block of S
    softmax block (f32, incl. double-exp of the reference), AllGather(scores)
    M1 = scores^T @ W1, h-sharded per core, AllGather(M1)  [x2 for q/k]
    MLP: hiddenT = relu(M1^T @ Vc^T + b1);  dom = hidden^T-contracted @ W2 + b2
outputs in natural [N, D] f32 orientation.
"""

import numpy as np
import ml_dtypes

N, D, H = 32768, 1024, 4096
NCORES = 8
NS = N // NCORES          # 4096 sample rows per core
HS = H // NCORES          # 512 hidden cols per core (M1 shard)
P = 128
BF = ml_dtypes.bfloat16

_CACHE: dict = {}


def _build():
    import concourse.bass as bass
    import concourse.tile as tile
    from concourse import bacc, mybir

    f32 = mybir.dt.float32
    bf16 = mybir.dt.bfloat16
    Exp = mybir.ActivationFunctionType.Exp
    add = mybir.AluOpType.add
    mx = mybir.AluOpType.max
    mult = mybir.AluOpType.mult

    nc = bacc.Bacc("TRN2", target_bir_lowering=False, debug=False, num_devices=NCORES)

    # ---- I/O ----
    q = nc.dram_tensor("q", [NS, D], bf16, kind="ExternalInput")
    k = nc.dram_tensor("k", [NS, D], bf16, kind="ExternalInput")
    vt = nc.dram_tensor("vt", [D, NS], bf16, kind="ExternalInput")
    w1s = {m: nc.dram_tensor(f"w1s_{m}", [D, HS], bf16, kind="ExternalInput") for m in "qk"}
    w2 = {m: nc.dram_tensor(f"w2_{m}", [H, D], bf16, kind="ExternalInput") for m in "qk"}
    b1t = {m: nc.dram_tensor(f"b1t_{m}", [P, H // P], f32, kind="ExternalInput") for m in "qk"}
    b2r = {m: nc.dram_tensor(f"b2r_{m}", [1, D], f32, kind="ExternalInput") for m in "qk"}
    mask = nc.dram_tensor("mask", [P, D], bf16, kind="ExternalInput")
    dom = {m: nc.dram_tensor(f"dom_{m}", [NS, D], f32, kind="ExternalOutput") for m in "qk"}

    # ---- internal DRAM (collective bounce buffers) ----
    s_part = [nc.dram_tensor(f"s_part{j}", [D, 512], bf16) for j in range(2)]
    s_red = [nc.dram_tensor(f"s_red{j}", [P, 512], bf16) for j in range(2)]
    scb = nc.dram_tensor("scb", [P, D], bf16)
    sc_full = nc.dram_tensor("sc_full", [D, D], bf16, addr_space="Shared")
    m1s = {(m, h): nc.dram_tensor(f"m1s_{m}{h}", [D, HS // 2], bf16)
           for m in "qk" for h in range(2)}
    m1f = {(m, h): nc.dram_tensor(f"m1f_{m}{h}", [NCORES, D, HS // 2], bf16,
                                  addr_space="Shared")
           for m in "qk" for h in range(2)}

    RG = [list(range(NCORES))]
    NB = NS // P              # 32 n-blocks per core
    IT = D // P               # 8 feature tiles
    JW = 512                  # matmul moving free dim
    JH = D // JW              # 2 j-halves of S
    HB = H // P               # 32 hidden blocks
    KO = 4                    # phase-1 k-stream chunks (of NB//KO n-blocks each)
    NBC = NB // KO            # 8 n-blocks per stream chunk

    with tile.TileContext(nc) as tc:
        with (
            tc.tile_pool(name="small", bufs=1) as small,
            tc.tile_pool(name="dout", bufs=4) as doutp,
            tc.tile_pool(name="wpool", bufs=1) as wpool,
        ):
            mask_sb = small.tile([P, D], bf16)
            w2_tiles = {}
            w2_tiles["q"] = wpool.tile([P, HB, D], bf16, tag="w2big", name="w2_q")

            # ================= phase 1: S_partial = Qc^T Kc =================
            smx_cm = tc.tile_pool(name="smx", bufs=1)
            smx = smx_cm.__enter__()
            e2h, zh = [], []
            with (
                tc.tile_pool(name="ph1", bufs=1) as ph1,
                tc.tile_pool(name="kstream", bufs=2) as kstream,
                tc.tile_pool(name="ph1psum", bufs=1, space="PSUM") as ph1psum,
            ):
                q_ch = {}
                for jh in range(JH):
                    ps = [
                        ph1psum.tile([P, JW], f32, tag=f"sps{i}", name=f"sps{i}_{jh}")
                        for i in range(IT)
                    ]
                    for ko in range(KO):
                        if ko not in q_ch:
                            qc = ph1.tile([P, NBC, D], bf16, tag=f"qc{ko}",
                                          name=f"qc{ko}")
                            nc.sync.dma_start(
                                out=qc[:],
                                in_=q.ap()[ko * NBC * P:(ko + 1) * NBC * P, :]
                                    .rearrange("(nb p) d -> p nb d", p=P),
                            )
                            q_ch[ko] = qc
                        k_sb = kstream.tile([P, NBC, JW], bf16, tag="kc")
                        nc.sync.dma_start(
                            out=k_sb[:],
                            in_=k.ap()[ko * NBC * P:(ko + 1) * NBC * P,
                                       jh * JW:(jh + 1) * JW]
                                .rearrange("(nb p) d -> p nb d", p=P),
                        )
                        # trickle-load mask + next MLP's w2 behind the
                        # phase-1 operand stream
                        idx = jh * KO + ko
                        if idx == 0:
                            nc.sync.dma_start(out=mask_sb[:], in_=mask.ap())
                        nc.sync.dma_start(
                            out=w2_tiles["q"][:, idx * (HB // 8):(idx + 1) * (HB // 8), :],
                            in_=w2["q"].ap()
                                .rearrange("(hb p) d -> p hb d", p=P)[
                                    :, idx * (HB // 8):(idx + 1) * (HB // 8), :],
                        )
                        for nb in range(NBC):
                            for i in range(IT):
                                nc.tensor.matmul(
                                    ps[i][:],
                                    q_ch[ko][:, nb, i * P:(i + 1) * P],
                                    k_sb[:, nb, :],
                                    start=(ko == 0 and nb == 0),
                                    stop=(ko == KO - 1 and nb == NBC - 1),
                                )
                    for i in range(IT):
                        so = doutp.tile([P, JW], bf16, tag="sout")
                        nc.vector.tensor_copy(out=so[:], in_=ps[i][:])
                        nc.sync.dma_start(
                            out=s_part[jh].ap()[i * P:(i + 1) * P, :],
                            in_=so[:],
                        )
                    # ReduceScatter this column-half; the jh=0 one overlaps
                    # the jh=1 matmuls.
                    nc.gpsimd.collective_compute(
                        "ReduceScatter", add, replica_groups=RG,
                        ins=[s_part[jh].ap().opt()], outs=[s_red[jh].ap().opt()],
                    )
                    # softmax front half: runs as soon as this RS lands,
                    # overlapping the other half's matmuls / RS.
                    sred = smx.tile([P, JW], bf16, tag=f"sred{jh}", name=f"sred{jh}")
                    nc.sync.dma_start(out=sred[:], in_=s_red[jh].ap())
                    tm = smx.tile([P, JW], f32, tag=f"tm{jh}", name=f"tm{jh}")
                    nc.vector.tensor_tensor(
                        out=tm[:], in0=sred[:],
                        in1=mask_sb[:, jh * JW:(jh + 1) * JW], op=mult)
                    lg = smx.tile([P, JW], f32, tag=f"lg{jh}", name=f"lg{jh}")
                    nc.scalar.activation(out=lg[:], in_=tm[:], func=Exp)
                    e2 = smx.tile([P, JW], f32, tag=f"e2{jh}", name=f"e2{jh}")
                    zz = smx.tile([P, 1], f32, tag=f"z{jh}", name=f"z{jh}")
                    nc.scalar.activation(out=e2[:], in_=lg[:], func=Exp,
                                         accum_out=zz[:])
                    e2h.append(e2)
                    zh.append(zz)

            # ================= softmax merge tail =================
            zsum = smx.tile([P, 1], f32)
            nc.vector.tensor_tensor(out=zsum[:], in0=zh[0][:], in1=zh[1][:], op=add)
            rz = smx.tile([P, 1], f32)
            nc.vector.reciprocal(rz[:], zsum[:])
            scb_sb = smx.tile([P, D], bf16)
            for j in range(2):
                nc.vector.tensor_scalar(out=scb_sb[:, j * JW:(j + 1) * JW],
                                        in0=e2h[j][:], scalar1=rz[:],
                                        scalar2=None, op0=mult)
            nc.sync.dma_start(out=scb.ap(), in_=scb_sb[:])
            smx_cm.__exit__(None, None, None)

            nc.gpsimd.collective_compute(
                "AllGather", mybir.AluOpType.bypass, replica_groups=RG,
                ins=[scb.ap().opt()], outs=[sc_full.ap().opt()],
            )

            # ================= M1 = scores^T @ W1 (h-shard) =================
            with (
                tc.tile_pool(name="m1pool", bufs=1) as m1pool,
                tc.tile_pool(name="m1psum", bufs=3, space="PSUM") as m1psum,
            ):
                sc_t = []
                for it in range(IT):
                    sct = m1pool.tile([P, D], bf16, tag=f"sc{it}", name=f"sc{it}")
                    nc.sync.dma_start(
                        out=sct[:],
                        in_=sc_full.ap()[it * P:(it + 1) * P, :],
                    )
                    sc_t.append(sct)
                for m in "qk":
                    w1_sb = m1pool.tile([P, IT, HS], bf16, tag=f"w1_{m}")
                    nc.sync.dma_start(
                        out=w1_sb[:],
                        in_=w1s[m].ap().rearrange("(it p) h -> p it h", p=P),
                    )
                    for jm in range(IT):
                        mp = m1psum.tile([P, HS], f32, tag="m1ps",
                                         name=f"mp_{m}{jm}")
                        for it in range(IT):
                            nc.tensor.matmul(
                                mp[:],
                                sc_t[it][:, jm * P:(jm + 1) * P],
                                w1_sb[:, it, :],
                                start=(it == 0),
                                stop=(it == IT - 1),
                            )
                        mo = doutp.tile([P, HS], bf16, tag="m1out",
                                        name=f"mo_{m}{jm}")
                        nc.vector.tensor_copy(out=mo[:], in_=mp[:])
                        for half in range(2):
                            nc.sync.dma_start(
                                out=m1s[m, half].ap()[jm * P:(jm + 1) * P, :],
                                in_=mo[:, half * (HS // 2):(half + 1) * (HS // 2)],
                            )
                    for half in range(2):
                        nc.gpsimd.collective_compute(
                            "AllGather", mybir.AluOpType.bypass, replica_groups=RG,
                            ins=[m1s[m, half].ap().opt()],
                            outs=[m1f[m, half].ap().opt()],
                        )

            # ================= MLPs =================
            with (
                tc.tile_pool(name="mlp", bufs=1) as mlp,
                tc.tile_pool(name="vstream", bufs=2) as vstream,
                tc.tile_pool(name="mlppsum", bufs=5, space="PSUM") as bpsum,
                tc.tile_pool(name="cpsum", bufs=3, space="PSUM") as cpsum,
            ):
                for m in "qk":
                    HH = HS // 2
                    m1_half = []
                    for half in range(2):
                        row = []
                        for c2 in range(NCORES):
                            mt = mlp.tile([P, IT, HH], bf16,
                                          tag=f"m1big{half}_{c2}",
                                          name=f"m1t{half}_{c2}_{m}")
                            nc.sync.dma_start(
                                out=mt[:],
                                in_=m1f[m, half].ap()[c2]
                                    .rearrange("(jb p) h -> p jb h", p=P),
                            )
                            row.append(mt)
                        m1_half.append(row)
                    hb_order = [hb for hb in range(HB) if (hb % 4) < 2] + \
                               [hb for hb in range(HB) if (hb % 4) >= 2]
                    if m in w2_tiles:
                        w2_sb = w2_tiles[m]
                    else:
                        w2_sb = wpool.tile([P, HB, D], bf16, tag="w2big",
                                           name=f"w2_{m}")
                        nc.sync.dma_start(
                            out=w2_sb[:],
                            in_=w2[m].ap().rearrange("(hb p) d -> p hb d", p=P),
                        )
                    b1_sb = small.tile([P, H // P], f32, tag="b1t")
                    nc.sync.dma_start(out=b1_sb[:], in_=b1t[m].ap())
                    b2_sb = small.tile([P, D], f32, tag="b2r")
                    b2_bcast = b2r[m].ap()
                    nc.sync.dma_start(
                        out=b2_sb[:],
                        in_=bass.AP(tensor=b2_bcast.tensor, offset=b2_bcast.offset,
                                    ap=[[0, P], *b2_bcast.ap[1:]]),
                    )

                    for ncnk in range(NS // JW):      # 8 chunks of 512 samples
                        vt_sb = vstream.tile([P, IT, JW], bf16, tag="vt")
                        nc.sync.dma_start(
                            out=vt_sb[:],
                            in_=vt.ap()[:, ncnk * JW:(ncnk + 1) * JW]
                                .rearrange("(jb p) n -> p jb n", p=P),
                        )
                        hid_sb = mlp.tile([P, HB, JW], bf16, tag="hid")
                        # hiddenT[h, n] = relu(sum_j M1[j,h] vT[j,n] + b1[h])
                        for hb in hb_order:
                            c2, pos = hb // 4, hb % 4
                            half, hh = pos // 2, pos % 2
                            pb = bpsum.tile([P, JW], f32, tag="psB")
                            for jb in range(IT):
                                nc.tensor.matmul(
                                    pb[:],
                                    m1_half[half][c2][:, jb, hh * P:(hh + 1) * P],
                                    vt_sb[:, jb, :],
                                    start=(jb == 0),
                                    stop=(jb == IT - 1),
                                )
                            nc.vector.tensor_scalar(
                                out=hid_sb[:, hb, :], in0=pb[:],
                                scalar1=b1_sb[:, hb:hb + 1], scalar2=0.0,
                                op0=add, op1=mx,
                            )
                        # dom[n, i2] = sum_h hidden[n,h] W2[h,i2] + b2[i2]
                        for ns in range(JW // P):     # 4 sample sub-tiles
                            for ih in range(JH):      # 2 output column halves
                                pc = cpsum.tile([P, JW], f32, tag="psC")
                                for hb in range(HB):
                                    nc.tensor.matmul(
                                        pc[:],
                                        hid_sb[:, hb, ns * P:(ns + 1) * P],
                                        w2_sb[:, hb, ih * JW:(ih + 1) * JW],
                                        start=(hb == 0), stop=(hb == HB - 1),
                                    )
                                do = doutp.tile([P, JW], f32, tag="dmout")
                                nc.vector.tensor_tensor(
                                    out=do[:], in0=pc[:],
                                    in1=b2_sb[:, ih * JW:(ih + 1) * JW],
                                    op=add,
                                )
                                nc.sync.dma_start(
                                    out=dom[m].ap()[
                                        ncnk * JW + ns * P:ncnk * JW + (ns + 1) * P,
                                        ih * JW:(ih + 1) * JW],
                                    in_=do[:],
                                )

    nc.compile()
    return nc


def _get_nc():
    if "nc" not in _CACHE:
        _CACHE["nc"] = _build()
    return _CACHE["nc"]


def _make_in_maps(inputs):
    query = np.asarray(inputs["query"])
    key = np.asarray(inputs["key"])
    value = np.asarray(inputs["value"])

    q_bf = query.astype(BF)
    k_bf = key.astype(BF)
    vt_bf = np.ascontiguousarray(value.T).astype(BF)          # [D, N]
    w1 = {"q": inputs["wq1"], "k": inputs["wk1"]}
    w2 = {"q": inputs["wq2"], "k": inputs["wk2"]}
    b1 = {"q": inputs["bq1"], "k": inputs["bk1"]}
    b2 = {"q": inputs["bq2"], "k": inputs["bk2"]}
    w1_bf = {m: np.asarray(w1[m]).astype(BF) for m in "qk"}
    w2_bf = {m: np.ascontiguousarray(np.asarray(w2[m]).astype(BF)) for m in "qk"}
    b1_t = {m: np.ascontiguousarray(
        np.asarray(b1[m]).astype(np.float32).reshape(H // P, P).T) for m in "qk"}
    b2_r = {m: np.asarray(b2[m]).astype(np.float32).reshape(1, D) for m in "qk"}

    in_maps = []
    diag = 1.0 - 1.0 / np.sqrt(D).astype(np.float32)
    for c in range(NCORES):
        msk = np.ones((P, D), np.float32)
        msk[np.arange(P), c * P + np.arange(P)] = diag
        msk = msk.astype(BF)
        im = {
            "q": np.ascontiguousarray(q_bf[c * NS:(c + 1) * NS]),
            "k": np.ascontiguousarray(k_bf[c * NS:(c + 1) * NS]),
            "vt": np.ascontiguousarray(vt_bf[:, c * NS:(c + 1) * NS]),
            "mask": msk,
        }
        for m in "qk":
            im[f"w1s_{m}"] = np.ascontiguousarray(
                w1_bf[m][:, c * HS:(c + 1) * HS])
            im[f"w2_{m}"] = w2_bf[m]
            im[f"b1t_{m}"] = b1_t[m]
            im[f"b2r_{m}"] = b2_r[m]
        in_maps.append(im)
    return in_maps


def _gather(results):
    dom_q = np.concatenate([results[c]["dom_q"] for c in range(NCORES)], axis=0)
    dom_k = np.concatenate([results[c]["dom_k"] for c in range(NCORES)], axis=0)
    return dom_q, dom_k


def _run(inputs, **kw):
    from concourse import bass_utils
    nc = _get_nc()
    in_maps = _make_in_maps(inputs)
    return bass_utils.run_bass_kernel_spmd(
        nc, in_maps, core_ids=list(range(NCORES)), **kw
    )


def kernel(**inputs):
    res = _run(inputs)
    return _gather(res.results)



# revision 5
# speedup vs baseline: 1.5050x; 1.5050x over previous
"""Trainium2 Bass kernel for nn_DomainAdaptation (feature attention + dual MLP).

Math (reference):
    S = Q^T K                        [D, D], contraction over N
    L = exp(S - S*I/sqrt(D))
    scores = softmax(L, axis=-1)
    attn = (scores @ V^T)^T          [N, D]
    dom_m = relu(attn @ Wm1 + bm1) @ Wm2 + bm2        for m in {q, k}

Restructuring: attn @ W1 = V @ (scores^T @ W1) = V @ M1, attn never
materialized.  fp8 DoubleRow matmuls with exact rank corrections:

  scores rows sum to 1  =>  colmean(M1) = colmean(W1) =: wbar  (host-known)
  M1 = 1*wbar^T + Delta,  Delta tiny (~2% of M1)  ->  fp8 at fine scale
  L1:  hidT = relu(mt*wbar^T + V@Delta + b1),  mt = rowsum(V) (host-exact)
       rank-1 term via a 1-row bf16 matmul PSUM init, V/Delta in fp8 DR
  L2:  hidden ~ relu(mt wbar^T) = mt+ wbar+^T + mt- wbar-^T  (rank 2)
       R := hidden - relu(mt wbar^T)  (tiny)  ->  fp8
       dom = R@W2_f8 + mt+ g+ + mt- g- + b2,   g+- = wbar+-^T @ W2 (host)
  All rank operands are bf16 with power-of-2 scales so the decomposition
  is numerically consistent; fp8 quantization noise scales with the small
  residuals, giving bf16-class accuracy at fp8 speed.

Per core (N sharded 8 ways):
    phase 1: S'_partial = (Q*s)^T (K*s) fp8 DR, ReduceScatter, softmax
             (descale folded into the diag mask), AllGather(scores)
    phase B: M1 = scores^T @ W1 (bf16), Delta = M1*SD - wbar', fp8,
             AllGather(Delta) h-sharded   [x2 for q/k]
    MLP: as above, n-chunked, all heavy matmuls fp8 DoubleRow
"""

import numpy as np
import ml_dtypes

N, D, H = 32768, 1024, 4096
NCORES = 8
NS = N // NCORES          # 4096 sample rows per core
HS = H // NCORES          # 512 hidden cols per core (Delta shard)
P = 128
JW = 512                  # matmul moving free dim
IT = D // P               # 8 feature tiles
HB = H // P               # 32 hidden blocks
NB = NS // P              # 32 n-blocks per core
KO = 4                    # phase-1 k-stream chunks
NBC = NB // KO            # 8 n-blocks per stream chunk
JH = D // JW              # 2 column halves of S

BF = ml_dtypes.bfloat16
F8 = ml_dtypes.float8_e4m3

SQK = 2048.0              # 2^11  q/k fp8 scale
SV = 2048.0               # 2^11  v fp8 scale
SD = 131072.0             # 2^17  Delta fp8 scale
SR = 262144.0             # 2^18  R fp8 scale
SW2 = 1024.0              # 2^10  W2 fp8 scale
C1 = SR / (SV * SD)       # PSUM1 -> hidden*SR
C2 = 1.0 / (SR * SW2)     # PSUM2 -> dom

_CACHE: dict = {}


def _build():
    import concourse.bass as bass
    import concourse.tile as tile
    from concourse import bacc, mybir

    f32 = mybir.dt.float32
    bf16 = mybir.dt.bfloat16
    f8 = mybir.dt.float8e4
    Exp = mybir.ActivationFunctionType.Exp
    Relu = mybir.ActivationFunctionType.Relu
    Copy = mybir.ActivationFunctionType.Copy
    DRm = mybir.MatmulPerfMode.DoubleRow
    add = mybir.AluOpType.add
    mx = mybir.AluOpType.max
    mult = mybir.AluOpType.mult
    sub = mybir.AluOpType.subtract

    nc = bacc.Bacc("TRN2", target_bir_lowering=False, debug=False, num_devices=NCORES)

    # ---- I/O ----
    q = nc.dram_tensor("q", [NS, D], f8, kind="ExternalInput")
    k = nc.dram_tensor("k", [NS, D], f8, kind="ExternalInput")
    vt = nc.dram_tensor("vt", [D, NS], f8, kind="ExternalInput")
    mtd = nc.dram_tensor("mt", [1, NS], bf16, kind="ExternalInput")
    mpm = nc.dram_tensor("mpm", [2, NS], bf16, kind="ExternalInput")
    mask = nc.dram_tensor("mask", [P, D], bf16, kind="ExternalInput")
    w1s = {m: nc.dram_tensor(f"w1s_{m}", [D, HS], bf16, kind="ExternalInput") for m in "qk"}
    w28 = {m: nc.dram_tensor(f"w28_{m}", [H, D], f8, kind="ExternalInput") for m in "qk"}
    wbp = {m: nc.dram_tensor(f"wbp_{m}", [1, H], bf16, kind="ExternalInput") for m in "qk"}
    wbl = {m: nc.dram_tensor(f"wbl_{m}", [1, HS], bf16, kind="ExternalInput") for m in "qk"}
    wbs = {m: nc.dram_tensor(f"wbs_{m}", [P, HB], f32, kind="ExternalInput") for m in "qk"}
    gg = {m: nc.dram_tensor(f"gg_{m}", [2, D], bf16, kind="ExternalInput") for m in "qk"}
    b1t = {m: nc.dram_tensor(f"b1t_{m}", [P, HB], f32, kind="ExternalInput") for m in "qk"}
    b2r = {m: nc.dram_tensor(f"b2r_{m}", [1, D], f32, kind="ExternalInput") for m in "qk"}
    dom = {m: nc.dram_tensor(f"dom_{m}", [NS, D], f32, kind="ExternalOutput") for m in "qk"}

    # ---- internal DRAM (collective bounce buffers) ----
    s_part = [nc.dram_tensor(f"s_part{j}", [D, JW], bf16) for j in range(JH)]
    s_red = [nc.dram_tensor(f"s_red{j}", [P, JW], bf16) for j in range(JH)]
    scb = nc.dram_tensor("scb", [P, D], bf16)
    sc_full = nc.dram_tensor("sc_full", [D, D], bf16, addr_space="Shared")
    d8s = {(m, h): nc.dram_tensor(f"d8s_{m}{h}", [D, HS // 2], f8)
           for m in "qk" for h in range(2)}
    d8f = {(m, h): nc.dram_tensor(f"d8f_{m}{h}", [NCORES, D, HS // 2], f8,
                                  addr_space="Shared")
           for m in "qk" for h in range(2)}

    RG = [list(range(NCORES))]

    with tile.TileContext(nc) as tc:
        with (
            tc.tile_pool(name="small", bufs=1) as small,
            tc.tile_pool(name="dout", bufs=4) as doutp,
            tc.tile_pool(name="wpool", bufs=1) as wpool,
        ):
            mask_sb = small.tile([P, D], bf16)
            mt_sb = small.tile([1, NS], bf16)
            mpm_sb = small.tile([2, NS], bf16)
            wbp_sb = {m: small.tile([1, H], bf16, tag=f"wbp{m}", name=f"wbp{m}") for m in "qk"}
            wbs_sb = {m: small.tile([P, HB], f32, tag=f"wbs{m}", name=f"wbs{m}") for m in "qk"}
            g_sb = {m: small.tile([2, D], bf16, tag=f"g{m}", name=f"g{m}") for m in "qk"}
            b1_sb = {m: small.tile([P, HB], f32, tag=f"b1{m}", name=f"b1{m}") for m in "qk"}
            b2_sb = {m: small.tile([P, D], f32, tag=f"b2{m}", name=f"b2{m}") for m in "qk"}
            w2_tiles = {"q": wpool.tile([P, HB, D], f8, tag="w2big", name="w2_q")}

            # ================= phase 1: S' = (Qs)^T (Ks), fp8 DR =============
            smx_cm = tc.tile_pool(name="smx", bufs=1)
            smx = smx_cm.__enter__()
            e2h, zh = [], []
            with (
                tc.tile_pool(name="ph1", bufs=1) as ph1,
                tc.tile_pool(name="kstream", bufs=2) as kstream,
                tc.tile_pool(name="ph1psum", bufs=1, space="PSUM") as ph1psum,
            ):
                q_ch = {}
                for jh in range(JH):
                    ps = [
                        ph1psum.tile([P, JW], f32, tag=f"sps{i}", name=f"sps{i}_{jh}")
                        for i in range(IT)
                    ]
                    for ko in range(KO):
                        if ko not in q_ch:
                            qc = ph1.tile([P, NBC, D], f8, tag=f"qc{ko}",
                                          name=f"qc{ko}")
                            nc.sync.dma_start(
                                out=qc[:],
                                in_=q.ap()[ko * NBC * P:(ko + 1) * NBC * P, :]
                                    .rearrange("(nb p) d -> p nb d", p=P),
                            )
                            q_ch[ko] = qc
                        k_sb = kstream.tile([P, NBC, JW], f8, tag="kc")
                        nc.sync.dma_start(
                            out=k_sb[:],
                            in_=k.ap()[ko * NBC * P:(ko + 1) * NBC * P,
                                       jh * JW:(jh + 1) * JW]
                                .rearrange("(nb p) d -> p nb d", p=P),
                        )
                        # trickle-load small tensors + W2(q) fp8 behind the
                        # phase-1 operand stream
                        idx = jh * KO + ko
                        if idx == 0:
                            nc.sync.dma_start(out=mask_sb[:], in_=mask.ap())
                            nc.sync.dma_start(out=mt_sb[:], in_=mtd.ap())
                            nc.sync.dma_start(out=mpm_sb[:], in_=mpm.ap())
                        if idx == 1:
                            for m in "qk":
                                nc.sync.dma_start(out=wbp_sb[m][:], in_=wbp[m].ap())
                                nc.sync.dma_start(out=wbs_sb[m][:], in_=wbs[m].ap())
                                nc.sync.dma_start(out=g_sb[m][:], in_=gg[m].ap())
                                nc.sync.dma_start(out=b1_sb[m][:], in_=b1t[m].ap())
                                bb = b2r[m].ap()
                                nc.sync.dma_start(
                                    out=b2_sb[m][:],
                                    in_=bass.AP(tensor=bb.tensor, offset=bb.offset,
                                                ap=[[0, P], *bb.ap[1:]]),
                                )
                        nc.sync.dma_start(
                            out=w2_tiles["q"][:, idx * (HB // 8):(idx + 1) * (HB // 8), :],
                            in_=w28["q"].ap()
                                .rearrange("(hb p) d -> p hb d", p=P)[
                                    :, idx * (HB // 8):(idx + 1) * (HB // 8), :],
                        )
                        for nb in range(0, NBC, 2):
                            for i in range(IT):
                                nc.tensor.matmul(
                                    ps[i][:],
                                    q_ch[ko][:, nb:nb + 2, i * P:(i + 1) * P],
                                    k_sb[:, nb:nb + 2, :],
                                    start=(ko == 0 and nb == 0),
                                    stop=(ko == KO - 1 and nb == NBC - 2),
                                    perf_mode=DRm,
                                )
                    for i in range(IT):
                        so = doutp.tile([P, JW], bf16, tag="sout")
                        nc.vector.tensor_copy(out=so[:], in_=ps[i][:])
                        nc.sync.dma_start(
                            out=s_part[jh].ap()[i * P:(i + 1) * P, :],
                            in_=so[:],
                        )
                    nc.gpsimd.collective_compute(
                        "ReduceScatter", add, replica_groups=RG,
                        ins=[s_part[jh].ap().opt()], outs=[s_red[jh].ap().opt()],
                    )
                    # softmax front half overlaps the other half's matmuls/RS
                    sred = smx.tile([P, JW], bf16, tag=f"sred{jh}", name=f"sred{jh}")
                    nc.sync.dma_start(out=sred[:], in_=s_red[jh].ap())
                    tm = smx.tile([P, JW], f32, tag=f"tm{jh}", name=f"tm{jh}")
                    nc.vector.tensor_tensor(
                        out=tm[:], in0=sred[:],
                        in1=mask_sb[:, jh * JW:(jh + 1) * JW], op=mult)
                    lg = smx.tile([P, JW], f32, tag=f"lg{jh}", name=f"lg{jh}")
                    nc.scalar.activation(out=lg[:], in_=tm[:], func=Exp)
                    e2 = smx.tile([P, JW], f32, tag=f"e2{jh}", name=f"e2{jh}")
                    zz = smx.tile([P, 1], f32, tag=f"z{jh}", name=f"z{jh}")
                    nc.scalar.activation(out=e2[:], in_=lg[:], func=Exp,
                                         accum_out=zz[:])
                    e2h.append(e2)
                    zh.append(zz)

            # ================= softmax merge tail =================
            zsum = smx.tile([P, 1], f32)
            nc.vector.tensor_tensor(out=zsum[:], in0=zh[0][:], in1=zh[1][:], op=add)
            rz = smx.tile([P, 1], f32)
            nc.vector.reciprocal(rz[:], zsum[:])
            scb_sb = smx.tile([P, D], bf16)
            for j in range(JH):
                nc.vector.tensor_scalar(out=scb_sb[:, j * JW:(j + 1) * JW],
                                        in0=e2h[j][:], scalar1=rz[:],
                                        scalar2=None, op0=mult)
            nc.sync.dma_start(out=scb.ap(), in_=scb_sb[:])
            smx_cm.__exit__(None, None, None)

            nc.gpsimd.collective_compute(
                "AllGather", mybir.AluOpType.bypass, replica_groups=RG,
                ins=[scb.ap().opt()], outs=[sc_full.ap().opt()],
            )

            # ====== phase B: M1 = scores^T @ W1, Delta = M1*SD - wbar' ======
            with (
                tc.tile_pool(name="m1pool", bufs=1) as m1pool,
                tc.tile_pool(name="m1psum", bufs=3, space="PSUM") as m1psum,
            ):
                sc_t = []
                for it in range(IT):
                    sct = m1pool.tile([P, D], bf16, tag=f"sc{it}", name=f"sc{it}")
                    nc.sync.dma_start(
                        out=sct[:],
                        in_=sc_full.ap()[it * P:(it + 1) * P, :],
                    )
                    sc_t.append(sct)
                for m in "qk":
                    w1_sb = m1pool.tile([P, IT, HS], bf16, tag=f"w1_{m}", name=f"w1_{m}")
                    nc.sync.dma_start(
                        out=w1_sb[:],
                        in_=w1s[m].ap().rearrange("(it p) h -> p it h", p=P),
                    )
                    wbl_bc = m1pool.tile([P, HS], bf16, tag=f"wblb_{m}", name=f"wblb_{m}")
                    wa = wbl[m].ap()
                    nc.sync.dma_start(
                        out=wbl_bc[:],
                        in_=bass.AP(tensor=wa.tensor, offset=wa.offset,
                                    ap=[[0, P], *wa.ap[1:]]),
                    )
                    for jm in range(IT):
                        mp = m1psum.tile([P, HS], f32, tag="m1ps",
                                         name=f"mp_{m}{jm}")
                        for it in range(IT):
                            nc.tensor.matmul(
                                mp[:],
                                sc_t[it][:, jm * P:(jm + 1) * P],
                                w1_sb[:, it, :],
                                start=(it == 0),
                                stop=(it == IT - 1),
                            )
                        dsub = doutp.tile([P, HS], f8, tag="m1d8",
                                          name=f"dsub_{m}{jm}")
                        nc.vector.scalar_tensor_tensor(
                            out=dsub[:], in0=mp[:], scalar=SD, in1=wbl_bc[:],
                            op0=mult, op1=sub)
                        for half in range(2):
                            nc.sync.dma_start(
                                out=d8s[m, half].ap()[jm * P:(jm + 1) * P, :],
                                in_=dsub[:, half * (HS // 2):(half + 1) * (HS // 2)],
                            )
                    for half in range(2):
                        nc.gpsimd.collective_compute(
                            "AllGather", mybir.AluOpType.bypass, replica_groups=RG,
                            ins=[d8s[m, half].ap().opt()],
                            outs=[d8f[m, half].ap().opt()],
                        )

            # ================= MLPs =================
            with (
                tc.tile_pool(name="mlp", bufs=1) as mlp,
                tc.tile_pool(name="vstream", bufs=2) as vstream,
                tc.tile_pool(name="tpool", bufs=3) as tpool,
                tc.tile_pool(name="mlppsum", bufs=4, space="PSUM") as bpsum,
                tc.tile_pool(name="cpsum", bufs=3, space="PSUM") as cpsum,
            ):
                for m in "qk":
                    HH = HS // 2
                    d_half = []
                    for half in range(2):
                        row = []
                        for c2 in range(NCORES):
                            mt2 = mlp.tile([P, IT, HH], f8,
                                           tag=f"d8big{half}_{c2}",
                                           name=f"d8t{half}_{c2}_{m}")
                            nc.sync.dma_start(
                                out=mt2[:],
                                in_=d8f[m, half].ap()[c2]
                                    .rearrange("(jb p) h -> p jb h", p=P),
                            )
                            row.append(mt2)
                        d_half.append(row)
                    hb_order = [hb for hb in range(HB) if (hb % 4) < 2] + \
                               [hb for hb in range(HB) if (hb % 4) >= 2]
                    if m in w2_tiles:
                        w2_sb = w2_tiles[m]
                    else:
                        w2_sb = wpool.tile([P, HB, D], f8, tag="w2big",
                                           name=f"w2_{m}")
                        nc.sync.dma_start(
                            out=w2_sb[:],
                            in_=w28[m].ap().rearrange("(hb p) d -> p hb d", p=P),
                        )

                    for ncnk in range(NS // JW):      # 8 chunks of 512 samples
                        vt_sb = vstream.tile([P, IT, JW], f8, tag="vt")
                        nc.sync.dma_start(
                            out=vt_sb[:],
                            in_=vt.ap()[:, ncnk * JW:(ncnk + 1) * JW]
                                .rearrange("(jb p) n -> p jb n", p=P),
                        )
                        mbc = vstream.tile([P, JW], bf16, tag="mbc")
                        ma = mtd.ap()
                        nc.sync.dma_start(
                            out=mbc[:],
                            in_=bass.AP(tensor=ma.tensor,
                                        offset=ma.offset + ncnk * JW,
                                        ap=[[0, P], [1, JW]]),
                        )
                        hid_sb = mlp.tile([P, HB, JW], f8, tag="hid")
                        # hidT[h,n] = relu(mt_n wbar_h + sum_j V[n,j]Delta[j,h] + b1)
                        for hb in hb_order:
                            c2, pos = hb // 4, hb % 4
                            half, hh = pos // 2, pos % 2
                            pb = bpsum.tile([P, JW], f32, tag="psB")
                            nc.tensor.matmul(
                                pb[:],
                                wbp_sb[m][0:1, hb * P:(hb + 1) * P],
                                mt_sb[0:1, ncnk * JW:(ncnk + 1) * JW],
                                start=True, stop=False,
                            )
                            for jb in range(0, IT, 2):
                                nc.tensor.matmul(
                                    pb[:],
                                    d_half[half][c2][:, jb:jb + 2,
                                                     hh * P:(hh + 1) * P],
                                    vt_sb[:, jb:jb + 2, :],
                                    start=False,
                                    stop=(jb == IT - 2),
                                    perf_mode=DRm,
                                )
                            t1 = tpool.tile([P, JW], f32, tag="t1")
                            nc.scalar.activation(out=t1[:], in_=pb[:], func=Relu,
                                                 scale=C1,
                                                 bias=b1_sb[m][:, hb:hb + 1])
                            t2 = tpool.tile([P, JW], f32, tag="t2")
                            nc.vector.tensor_scalar(
                                out=t2[:], in0=mbc[:],
                                scalar1=wbs_sb[m][:, hb:hb + 1], scalar2=0.0,
                                op0=mult, op1=mx,
                            )
                            nc.vector.tensor_tensor(
                                out=hid_sb[:, hb, :], in0=t1[:], in1=t2[:],
                                op=sub,
                            )
                        # dom[n,i2] = R@W2*(SR*SW2) + mt+- g+- + b2
                        for ns in range(JW // P):     # 4 sample sub-tiles
                            for ih in range(JH):      # 2 output column halves
                                pc = cpsum.tile([P, JW], f32, tag="psC")
                                nc.tensor.matmul(
                                    pc[:],
                                    mpm_sb[:, ncnk * JW + ns * P:
                                           ncnk * JW + (ns + 1) * P],
                                    g_sb[m][:, ih * JW:(ih + 1) * JW],
                                    start=True, stop=False,
                                )
                                for hb in range(0, HB, 2):
                                    nc.tensor.matmul(
                                        pc[:],
                                        hid_sb[:, hb:hb + 2, ns * P:(ns + 1) * P],
                                        w2_sb[:, hb:hb + 2, ih * JW:(ih + 1) * JW],
                                        start=False, stop=(hb == HB - 2),
                                        perf_mode=DRm,
                                    )
                                do = doutp.tile([P, JW], f32, tag="dmout")
                                nc.vector.scalar_tensor_tensor(
                                    out=do[:], in0=pc[:], scalar=C2,
                                    in1=b2_sb[m][:, ih * JW:(ih + 1) * JW],
                                    op0=mult, op1=add,
                                )
                                nc.sync.dma_start(
                                    out=dom[m].ap()[
                                        ncnk * JW + ns * P:ncnk * JW + (ns + 1) * P,
                                        ih * JW:(ih + 1) * JW],
                                    in_=do[:],
                                )

    nc.compile()
    return nc


def _get_nc():
    if "nc" not in _CACHE:
        _CACHE["nc"] = _build()
    return _CACHE["nc"]


def _make_in_maps(inputs):
    query = np.asarray(inputs["query"], np.float32)
    key = np.asarray(inputs["key"], np.float32)
    value = np.asarray(inputs["value"], np.float32)

    q_f8 = (query * SQK).astype(F8)
    k_f8 = (key * SQK).astype(F8)
    vt_f8 = (np.ascontiguousarray(value.T) * SV).astype(F8)       # [D, N]

    # mt = rowsum(V) exact; bf16 value consistency via power-of-2 scales
    mt_bf = (value.astype(np.float64).sum(axis=1) * SV).astype(BF)   # [N]
    m_real = mt_bf.astype(np.float64) / SV
    mpm_bf = np.stack([np.maximum(m_real, 0.0),
                       np.maximum(-m_real, 0.0)]).astype(BF)         # [2, N]

    w1 = {"q": np.asarray(inputs["wq1"], np.float32),
          "k": np.asarray(inputs["wk1"], np.float32)}
    w2 = {"q": np.asarray(inputs["wq2"], np.float32),
          "k": np.asarray(inputs["wk2"], np.float32)}
    b1 = {"q": np.asarray(inputs["bq1"], np.float32),
          "k": np.asarray(inputs["bk1"], np.float32)}
    b2 = {"q": np.asarray(inputs["bq2"], np.float32),
          "k": np.asarray(inputs["bk2"], np.float32)}

    w1_bf, w28_, wbp_, wbs_, gg_, b1_, b2_ = {}, {}, {}, {}, {}, {}, {}
    for m in "qk":
        w1_bf[m] = w1[m].astype(BF)
        w28_[m] = np.ascontiguousarray(w2[m] * SW2).astype(F8)
        wbar = w1[m].astype(np.float64).mean(axis=0)                 # [H]
        wb_bf = (wbar * SD).astype(BF)                               # wbar' bf16
        wbp_[m] = wb_bf.reshape(1, H)
        wreal = wb_bf.astype(np.float64) / SD
        # wbs = wbar * SR / SV, exact scaling of the bf16 wbar' values
        wbs_[m] = np.ascontiguousarray(
            (wb_bf.astype(np.float32) * np.float32(C1))
            .reshape(HB, P).T).astype(np.float32)                    # [P, HB]
        gp = np.maximum(wreal, 0.0) @ w2[m].astype(np.float64)
        gm = np.maximum(-wreal, 0.0) @ w2[m].astype(np.float64)
        gg_[m] = (np.stack([gp, gm]) * (SR * SW2)).astype(BF)        # [2, D]
        b1_[m] = np.ascontiguousarray(
            (b1[m] * SR).astype(np.float32).reshape(HB, P).T)        # [P, HB]
        b2_[m] = b2[m].astype(np.float32).reshape(1, D)

    diag = 1.0 - 1.0 / np.sqrt(np.float64(D))
    cmask = 1.0 / (SQK * SQK)
    in_maps = []
    for c in range(NCORES):
        msk = np.full((P, D), cmask, np.float64)
        msk[np.arange(P), c * P + np.arange(P)] = diag * cmask
        im = {
            "q": np.ascontiguousarray(q_f8[c * NS:(c + 1) * NS]),
            "k": np.ascontiguousarray(k_f8[c * NS:(c + 1) * NS]),
            "vt": np.ascontiguousarray(vt_f8[:, c * NS:(c + 1) * NS]),
            "mt": np.ascontiguousarray(mt_bf[c * NS:(c + 1) * NS]).reshape(1, NS),
            "mpm": np.ascontiguousarray(mpm_bf[:, c * NS:(c + 1) * NS]),
            "mask": msk.astype(BF),
        }
        for m in "qk":
            im[f"w1s_{m}"] = np.ascontiguousarray(w1_bf[m][:, c * HS:(c + 1) * HS])
            im[f"w28_{m}"] = w28_[m]
            im[f"wbp_{m}"] = wbp_[m]
            im[f"wbl_{m}"] = np.ascontiguousarray(
                wbp_[m][:, c * HS:(c + 1) * HS])
            im[f"wbs_{m}"] = wbs_[m]
            im[f"gg_{m}"] = gg_[m]
            im[f"b1t_{m}"] = b1_[m]
            im[f"b2r_{m}"] = b2_[m]
        in_maps.append(im)
    return in_maps


def _gather(results):
    dom_q = np.concatenate([results[c]["dom_q"] for c in range(NCORES)], axis=0)
    dom_k = np.concatenate([results[c]["dom_k"] for c in range(NCORES)], axis=0)
    return dom_q, dom_k


def _run(inputs, **kw):
    from concourse import bass_utils
    nc = _get_nc()
    in_maps = _make_in_maps(inputs)
    return bass_utils.run_bass_kernel_spmd(
        nc, in_maps, core_ids=list(range(NCORES)), **kw
    )


def kernel(**inputs):
    res = _run(inputs)
    return _gather(res.results)


# revision 7
# speedup vs baseline: 1.5213x; 1.0108x over previous
"""Trainium2 Bass kernel for nn_DomainAdaptation (feature attention + dual MLP).

Math (reference):
    S = Q^T K                        [D, D], contraction over N
    L = exp(S - S*I/sqrt(D))
    scores = softmax(L, axis=-1)
    attn = (scores @ V^T)^T          [N, D]
    dom_m = relu(attn @ Wm1 + bm1) @ Wm2 + bm2        for m in {q, k}

Restructuring: attn @ W1 = V @ (scores^T @ W1) = V @ M1, attn never
materialized.  fp8 DoubleRow matmuls with exact rank corrections:

  scores rows sum to 1  =>  colmean(M1) = colmean(W1) =: wbar  (host-known)
  M1 = 1*wbar^T + Delta,  Delta tiny (~2% of M1)  ->  fp8 at fine scale
  L1:  hidT = relu(mt*wbar^T + V@Delta + b1),  mt = rowsum(V) (host-exact)
       rank-1 term via a 1-row bf16 matmul PSUM init, V/Delta in fp8 DR
  L2:  hidden ~ relu(mt wbar^T) = mt+ wbar+^T + mt- wbar-^T  (rank 2)
       R := hidden - relu(mt wbar^T)  (tiny)  ->  fp8
       dom = R@W2_f8 + mt+ g+ + mt- g- + b2,   g+- = wbar+-^T @ W2 (host)
  All rank operands are bf16 with power-of-2 scales so the decomposition
  is numerically consistent; fp8 quantization noise scales with the small
  residuals, giving bf16-class accuracy at fp8 speed.

Per core (N sharded 8 ways):
    phase 1: S'_partial = (Q*s)^T (K*s) fp8 DR, ReduceScatter, softmax
             (descale folded into the diag mask), AllGather(scores)
    phase B: M1 = scores^T @ W1 (bf16), Delta = M1*SD - wbar', fp8,
             AllGather(Delta) h-sharded   [x2 for q/k]
    MLP: as above, n-chunked, all heavy matmuls fp8 DoubleRow
"""

import numpy as np
import ml_dtypes

N, D, H = 32768, 1024, 4096
NCORES = 8
NS = N // NCORES          # 4096 sample rows per core
HS = H // NCORES          # 512 hidden cols per core (Delta shard)
P = 128
JW = 512                  # matmul moving free dim
IT = D // P               # 8 feature tiles
HB = H // P               # 32 hidden blocks
NB = NS // P              # 32 n-blocks per core
KO = 4                    # phase-1 k-stream chunks
NBC = NB // KO            # 8 n-blocks per stream chunk
JH = D // JW              # 2 column halves of S

BF = ml_dtypes.bfloat16
F8 = ml_dtypes.float8_e4m3

SQK = 2048.0              # 2^11  q/k fp8 scale
SV = 2048.0               # 2^11  v fp8 scale
SD = 131072.0             # 2^17  Delta fp8 scale
SR = 262144.0             # 2^18  R fp8 scale
SW2 = 1024.0              # 2^10  W2 fp8 scale
C1 = SR / (SV * SD)       # PSUM1 -> hidden*SR
C2 = 1.0 / (SR * SW2)     # PSUM2 -> dom
FOUT = 1024.0             # 2^10  fp16 output scale (host divides)

_CACHE: dict = {}


def _build():
    import concourse.bass as bass
    import concourse.tile as tile
    from concourse import bacc, mybir

    f32 = mybir.dt.float32
    f16 = mybir.dt.float16
    bf16 = mybir.dt.bfloat16
    f8 = mybir.dt.float8e4
    Exp = mybir.ActivationFunctionType.Exp
    Relu = mybir.ActivationFunctionType.Relu
    Copy = mybir.ActivationFunctionType.Copy
    DRm = mybir.MatmulPerfMode.DoubleRow
    add = mybir.AluOpType.add
    mx = mybir.AluOpType.max
    mult = mybir.AluOpType.mult
    sub = mybir.AluOpType.subtract

    nc = bacc.Bacc("TRN2", target_bir_lowering=False, debug=False, num_devices=NCORES)

    # ---- I/O ----
    q = nc.dram_tensor("q", [NS, D], f8, kind="ExternalInput")
    k = nc.dram_tensor("k", [NS, D], f8, kind="ExternalInput")
    vt = nc.dram_tensor("vt", [D, NS], f8, kind="ExternalInput")
    mtd = nc.dram_tensor("mt", [1, NS], bf16, kind="ExternalInput")
    mpm = nc.dram_tensor("mpm", [2, NS], bf16, kind="ExternalInput")
    mask = nc.dram_tensor("mask", [P, D], bf16, kind="ExternalInput")
    w1s = {m: nc.dram_tensor(f"w1s_{m}", [D, HS], bf16, kind="ExternalInput") for m in "qk"}
    w28 = {m: nc.dram_tensor(f"w28_{m}", [H, D], f8, kind="ExternalInput") for m in "qk"}
    wbp = {m: nc.dram_tensor(f"wbp_{m}", [1, H], bf16, kind="ExternalInput") for m in "qk"}
    wbl = {m: nc.dram_tensor(f"wbl_{m}", [1, HS], bf16, kind="ExternalInput") for m in "qk"}
    wbs = {m: nc.dram_tensor(f"wbs_{m}", [P, HB], f32, kind="ExternalInput") for m in "qk"}
    gg = {m: nc.dram_tensor(f"gg_{m}", [2, D], bf16, kind="ExternalInput") for m in "qk"}
    b1t = {m: nc.dram_tensor(f"b1t_{m}", [P, HB], f32, kind="ExternalInput") for m in "qk"}
    b2r = {m: nc.dram_tensor(f"b2r_{m}", [1, D], f32, kind="ExternalInput") for m in "qk"}
    dom = {m: nc.dram_tensor(f"dom_{m}", [NS, D], f16, kind="ExternalOutput") for m in "qk"}

    # ---- internal DRAM (collective bounce buffers) ----
    s_part = [nc.dram_tensor(f"s_part{j}", [D, JW], bf16) for j in range(JH)]
    s_red = [nc.dram_tensor(f"s_red{j}", [P, JW], bf16) for j in range(JH)]
    scb = nc.dram_tensor("scb", [P, D], bf16)
    sc_full = nc.dram_tensor("sc_full", [D, D], bf16, addr_space="Shared")
    d8s = {(m, h): nc.dram_tensor(f"d8s_{m}{h}", [D, HS // 2], f8)
           for m in "qk" for h in range(2)}
    d8f = {(m, h): nc.dram_tensor(f"d8f_{m}{h}", [NCORES, D, HS // 2], f8,
                                  addr_space="Shared")
           for m in "qk" for h in range(2)}

    RG = [list(range(NCORES))]

    with tile.TileContext(nc) as tc:
        with (
            tc.tile_pool(name="small", bufs=1) as small,
            tc.tile_pool(name="dout", bufs=4) as doutp,
            tc.tile_pool(name="wpool", bufs=1) as wpool,
        ):
            mask_sb = small.tile([P, D], bf16)
            mt_sb = small.tile([1, NS], bf16)
            mpm_sb = small.tile([2, NS], bf16)
            wbp_sb = {m: small.tile([1, H], bf16, tag=f"wbp{m}", name=f"wbp{m}") for m in "qk"}
            wbs_sb = {m: small.tile([P, HB], f32, tag=f"wbs{m}", name=f"wbs{m}") for m in "qk"}
            g_sb = {m: small.tile([2, D], bf16, tag=f"g{m}", name=f"g{m}") for m in "qk"}
            b1_sb = {m: small.tile([P, HB], f32, tag=f"b1{m}", name=f"b1{m}") for m in "qk"}
            b2_sb = {m: small.tile([P, D], f32, tag=f"b2{m}", name=f"b2{m}") for m in "qk"}
            w2_tiles = {"q": wpool.tile([P, HB, D], f8, tag="w2big", name="w2_q")}

            # ================= phase 1: S' = (Qs)^T (Ks), fp8 DR =============
            smx_cm = tc.tile_pool(name="smx", bufs=1)
            smx = smx_cm.__enter__()
            e2h, zh = [], []
            with (
                tc.tile_pool(name="ph1", bufs=1) as ph1,
                tc.tile_pool(name="kstream", bufs=2) as kstream,
                tc.tile_pool(name="ph1psum", bufs=1, space="PSUM") as ph1psum,
            ):
                q_ch = {}
                for jh in range(JH):
                    ps = [
                        ph1psum.tile([P, JW], f32, tag=f"sps{i}", name=f"sps{i}_{jh}")
                        for i in range(IT)
                    ]
                    for ko in range(KO):
                        if ko not in q_ch:
                            qc = ph1.tile([P, NBC, D], f8, tag=f"qc{ko}",
                                          name=f"qc{ko}")
                            nc.sync.dma_start(
                                out=qc[:],
                                in_=q.ap()[ko * NBC * P:(ko + 1) * NBC * P, :]
                                    .rearrange("(nb p) d -> p nb d", p=P),
                            )
                            q_ch[ko] = qc
                        k_sb = kstream.tile([P, NBC, JW], f8, tag="kc")
                        nc.sync.dma_start(
                            out=k_sb[:],
                            in_=k.ap()[ko * NBC * P:(ko + 1) * NBC * P,
                                       jh * JW:(jh + 1) * JW]
                                .rearrange("(nb p) d -> p nb d", p=P),
                        )
                        # trickle-load small tensors + W2(q) fp8 behind the
                        # phase-1 operand stream
                        idx = jh * KO + ko
                        if idx == 0:
                            nc.sync.dma_start(out=mask_sb[:], in_=mask.ap())
                            nc.sync.dma_start(out=mt_sb[:], in_=mtd.ap())
                            nc.sync.dma_start(out=mpm_sb[:], in_=mpm.ap())
                        if idx == 1:
                            for m in "qk":
                                nc.sync.dma_start(out=wbp_sb[m][:], in_=wbp[m].ap())
                                nc.sync.dma_start(out=wbs_sb[m][:], in_=wbs[m].ap())
                                nc.sync.dma_start(out=g_sb[m][:], in_=gg[m].ap())
                                nc.sync.dma_start(out=b1_sb[m][:], in_=b1t[m].ap())
                                bb = b2r[m].ap()
                                nc.sync.dma_start(
                                    out=b2_sb[m][:],
                                    in_=bass.AP(tensor=bb.tensor, offset=bb.offset,
                                                ap=[[0, P], *bb.ap[1:]]),
                                )
                        nc.sync.dma_start(
                            out=w2_tiles["q"][:, idx * (HB // 8):(idx + 1) * (HB // 8), :],
                            in_=w28["q"].ap()
                                .rearrange("(hb p) d -> p hb d", p=P)[
                                    :, idx * (HB // 8):(idx + 1) * (HB // 8), :],
                        )
                        for nb in range(0, NBC, 2):
                            for i in range(IT):
                                nc.tensor.matmul(
                                    ps[i][:],
                                    q_ch[ko][:, nb:nb + 2, i * P:(i + 1) * P],
                                    k_sb[:, nb:nb + 2, :],
                                    start=(ko == 0 and nb == 0),
                                    stop=(ko == KO - 1 and nb == NBC - 2),
                                    perf_mode=DRm,
                                )
                    for i in range(IT):
                        so = doutp.tile([P, JW], bf16, tag="sout")
                        nc.vector.tensor_copy(out=so[:], in_=ps[i][:])
                        nc.sync.dma_start(
                            out=s_part[jh].ap()[i * P:(i + 1) * P, :],
                            in_=so[:],
                        )
                    nc.gpsimd.collective_compute(
                        "ReduceScatter", add, replica_groups=RG,
                        ins=[s_part[jh].ap().opt()], outs=[s_red[jh].ap().opt()],
                    )
                    # softmax front half overlaps the other half's matmuls/RS
                    sred = smx.tile([P, JW], bf16, tag=f"sred{jh}", name=f"sred{jh}")
                    nc.sync.dma_start(out=sred[:], in_=s_red[jh].ap())
                    tm = smx.tile([P, JW], f32, tag=f"tm{jh}", name=f"tm{jh}")
                    nc.vector.tensor_tensor(
                        out=tm[:], in0=sred[:],
                        in1=mask_sb[:, jh * JW:(jh + 1) * JW], op=mult)
                    lg = smx.tile([P, JW], f32, tag=f"lg{jh}", name=f"lg{jh}")
                    nc.scalar.activation(out=lg[:], in_=tm[:], func=Exp)
                    e2 = smx.tile([P, JW], f32, tag=f"e2{jh}", name=f"e2{jh}")
                    zz = smx.tile([P, 1], f32, tag=f"z{jh}", name=f"z{jh}")
                    nc.scalar.activation(out=e2[:], in_=lg[:], func=Exp,
                                         accum_out=zz[:])
                    e2h.append(e2)
                    zh.append(zz)

            # ================= softmax merge tail =================
            zsum = smx.tile([P, 1], f32)
            nc.vector.tensor_tensor(out=zsum[:], in0=zh[0][:], in1=zh[1][:], op=add)
            rz = smx.tile([P, 1], f32)
            nc.vector.reciprocal(rz[:], zsum[:])
            scb_sb = smx.tile([P, D], bf16)
            for j in range(JH):
                nc.vector.tensor_scalar(out=scb_sb[:, j * JW:(j + 1) * JW],
                                        in0=e2h[j][:], scalar1=rz[:],
                                        scalar2=None, op0=mult)
            nc.sync.dma_start(out=scb.ap(), in_=scb_sb[:])
            smx_cm.__exit__(None, None, None)

            nc.gpsimd.collective_compute(
                "AllGather", mybir.AluOpType.bypass, replica_groups=RG,
                ins=[scb.ap().opt()], outs=[sc_full.ap().opt()],
            )

            # ====== phase B: M1 = scores^T @ W1, Delta = M1*SD - wbar' ======
            with (
                tc.tile_pool(name="m1pool", bufs=1) as m1pool,
                tc.tile_pool(name="m1psum", bufs=3, space="PSUM") as m1psum,
            ):
                sc_t = []
                for it in range(IT):
                    sct = m1pool.tile([P, D], bf16, tag=f"sc{it}", name=f"sc{it}")
                    nc.sync.dma_start(
                        out=sct[:],
                        in_=sc_full.ap()[it * P:(it + 1) * P, :],
                    )
                    sc_t.append(sct)
                for m in "qk":
                    w1_sb = m1pool.tile([P, IT, HS], bf16, tag=f"w1_{m}", name=f"w1_{m}")
                    nc.sync.dma_start(
                        out=w1_sb[:],
                        in_=w1s[m].ap().rearrange("(it p) h -> p it h", p=P),
                    )
                    wbl_bc = m1pool.tile([P, HS], bf16, tag=f"wblb_{m}", name=f"wblb_{m}")
                    wa = wbl[m].ap()
                    nc.sync.dma_start(
                        out=wbl_bc[:],
                        in_=bass.AP(tensor=wa.tensor, offset=wa.offset,
                                    ap=[[0, P], *wa.ap[1:]]),
                    )
                    for jm in range(IT):
                        mp = m1psum.tile([P, HS], f32, tag="m1ps",
                                         name=f"mp_{m}{jm}")
                        for it in range(IT):
                            nc.tensor.matmul(
                                mp[:],
                                sc_t[it][:, jm * P:(jm + 1) * P],
                                w1_sb[:, it, :],
                                start=(it == 0),
                                stop=(it == IT - 1),
                            )
                        dsub = doutp.tile([P, HS], f8, tag="m1d8",
                                          name=f"dsub_{m}{jm}")
                        nc.vector.scalar_tensor_tensor(
                            out=dsub[:], in0=mp[:], scalar=SD, in1=wbl_bc[:],
                            op0=mult, op1=sub)
                        for half in range(2):
                            nc.sync.dma_start(
                                out=d8s[m, half].ap()[jm * P:(jm + 1) * P, :],
                                in_=dsub[:, half * (HS // 2):(half + 1) * (HS // 2)],
                            )
                    for half in range(2):
                        nc.gpsimd.collective_compute(
                            "AllGather", mybir.AluOpType.bypass, replica_groups=RG,
                            ins=[d8s[m, half].ap().opt()],
                            outs=[d8f[m, half].ap().opt()],
                        )

            # ================= MLPs =================
            NCK = NS // JW                # 8 chunks of 512 samples
            with (
                tc.tile_pool(name="mlp", bufs=1) as mlp,
                tc.tile_pool(name="vtres", bufs=1) as vtres,
                tc.tile_pool(name="vstream", bufs=2) as vstream,
                tc.tile_pool(name="tpool", bufs=3) as tpool,
                tc.tile_pool(name="mlppsum", bufs=4, space="PSUM") as bpsum,
                tc.tile_pool(name="cpsum", bufs=3, space="PSUM") as cpsum,
            ):
                hb_order = [hb for hb in range(HB) if (hb % 4) < 2] + \
                           [hb for hb in range(HB) if (hb % 4) >= 2]
                vt_tiles = {}
                for m in "qk":
                    HH = HS // 2
                    d_half = []
                    for half in range(2):
                        row = []
                        for c2 in range(NCORES):
                            mt2 = mlp.tile([P, IT, HH], f8,
                                           tag=f"d8big{half}_{c2}",
                                           name=f"d8t{half}_{c2}_{m}")
                            nc.sync.dma_start(
                                out=mt2[:],
                                in_=d8f[m, half].ap()[c2]
                                    .rearrange("(jb p) h -> p jb h", p=P),
                            )
                            row.append(mt2)
                        d_half.append(row)
                    if m in w2_tiles:
                        w2_sb = w2_tiles[m]
                    else:
                        w2_sb = wpool.tile([P, HB, D], f8, tag="w2big",
                                           name=f"w2_{m}")
                        nc.sync.dma_start(
                            out=w2_sb[:],
                            in_=w28[m].ap().rearrange("(hb p) d -> p hb d", p=P),
                        )

                    hid_db = [mlp.tile([P, HB, JW], f8, tag=f"hid{j}",
                                       name=f"hid{j}_{m}") for j in range(2)]

                    def load_vt(ncnk):
                        if ncnk in vt_tiles:
                            return
                        vt_sb = vtres.tile([P, IT, JW], f8, tag=f"vt{ncnk}",
                                           name=f"vt{ncnk}")
                        nc.sync.dma_start(
                            out=vt_sb[:],
                            in_=vt.ap()[:, ncnk * JW:(ncnk + 1) * JW]
                                .rearrange("(jb p) n -> p jb n", p=P),
                        )
                        vt_tiles[ncnk] = vt_sb

                    def do_l1(ncnk):
                        load_vt(ncnk)
                        if ncnk + 1 < NCK:
                            load_vt(ncnk + 1)
                        mbc = vstream.tile([P, JW], bf16, tag="mbc",
                                           name=f"mbc{m}{ncnk}")
                        ma = mtd.ap()
                        nc.sync.dma_start(
                            out=mbc[:],
                            in_=bass.AP(tensor=ma.tensor,
                                        offset=ma.offset + ncnk * JW,
                                        ap=[[0, P], [1, JW]]),
                        )
                        hid_sb = hid_db[ncnk % 2]
                        vt_sb = vt_tiles[ncnk]
                        # hidT[h,n] = relu(mt_n wbar_h + sum_j V[n,j]Delta[j,h] + b1)
                        for hb in hb_order:
                            c2, pos = hb // 4, hb % 4
                            half, hh = pos // 2, pos % 2
                            pb = bpsum.tile([P, JW], f32, tag="psB",
                                            name=f"psB{m}{ncnk}_{hb}")
                            nc.tensor.matmul(
                                pb[:],
                                wbp_sb[m][0:1, hb * P:(hb + 1) * P],
                                mt_sb[0:1, ncnk * JW:(ncnk + 1) * JW],
                                start=True, stop=False,
                            )
                            for jb in range(0, IT, 2):
                                nc.tensor.matmul(
                                    pb[:],
                                    d_half[half][c2][:, jb:jb + 2,
                                                     hh * P:(hh + 1) * P],
                                    vt_sb[:, jb:jb + 2, :],
                                    start=False,
                                    stop=(jb == IT - 2),
                                    perf_mode=DRm,
                                )
                            t1 = tpool.tile([P, JW], bf16, tag="t1",
                                            name=f"t1{m}{ncnk}_{hb}")
                            nc.scalar.activation(out=t1[:], in_=pb[:], func=Relu,
                                                 scale=C1,
                                                 bias=b1_sb[m][:, hb:hb + 1])
                            t2 = tpool.tile([P, JW], bf16, tag="t2",
                                            name=f"t2{m}{ncnk}_{hb}")
                            nc.vector.tensor_scalar(
                                out=t2[:], in0=mbc[:],
                                scalar1=wbs_sb[m][:, hb:hb + 1], scalar2=0.0,
                                op0=mult, op1=mx,
                            )
                            nc.vector.tensor_tensor(
                                out=hid_sb[:, hb, :], in0=t1[:], in1=t2[:],
                                op=sub,
                            )

                    def do_l2(ncnk):
                        hid_sb = hid_db[ncnk % 2]
                        # dom[n,i2] = R@W2*(SR*SW2) + mt+- g+- + b2
                        for ns in range(JW // P):     # 4 sample sub-tiles
                            for ih in range(JH):      # 2 output column halves
                                pc = cpsum.tile([P, JW], f32, tag="psC",
                                                name=f"psC{m}{ncnk}_{ns}{ih}")
                                nc.tensor.matmul(
                                    pc[:],
                                    mpm_sb[:, ncnk * JW + ns * P:
                                           ncnk * JW + (ns + 1) * P],
                                    g_sb[m][:, ih * JW:(ih + 1) * JW],
                                    start=True, stop=False,
                                )
                                for hb in range(0, HB, 2):
                                    nc.tensor.matmul(
                                        pc[:],
                                        hid_sb[:, hb:hb + 2, ns * P:(ns + 1) * P],
                                        w2_sb[:, hb:hb + 2, ih * JW:(ih + 1) * JW],
                                        start=False, stop=(hb == HB - 2),
                                        perf_mode=DRm,
                                    )
                                do = doutp.tile([P, JW], f16, tag="dmout",
                                                name=f"do{m}{ncnk}_{ns}{ih}")
                                nc.vector.scalar_tensor_tensor(
                                    out=do[:], in0=pc[:], scalar=C2 * FOUT,
                                    in1=b2_sb[m][:, ih * JW:(ih + 1) * JW],
                                    op0=mult, op1=add,
                                )
                                nc.sync.dma_start(
                                    out=dom[m].ap()[
                                        ncnk * JW + ns * P:ncnk * JW + (ns + 1) * P,
                                        ih * JW:(ih + 1) * JW],
                                    in_=do[:],
                                )

                    # software pipeline: L1(i+1) fills the PE while the DVE
                    # tail of L1(i) finishes producing hid(i)
                    do_l1(0)
                    for ncnk in range(1, NCK):
                        do_l1(ncnk)
                        do_l2(ncnk - 1)
                    do_l2(NCK - 1)

    nc.compile()
    return nc


def _get_nc():
    if "nc" not in _CACHE:
        _CACHE["nc"] = _build()
    return _CACHE["nc"]


def _make_in_maps(inputs):
    query = np.asarray(inputs["query"], np.float32)
    key = np.asarray(inputs["key"], np.float32)
    value = np.asarray(inputs["value"], np.float32)

    q_f8 = (query * SQK).astype(F8)
    k_f8 = (key * SQK).astype(F8)
    vt_f8 = (np.ascontiguousarray(value.T) * SV).astype(F8)       # [D, N]

    # mt = rowsum(V) exact; bf16 value consistency via power-of-2 scales
    mt_bf = (value.astype(np.float64).sum(axis=1) * SV).astype(BF)   # [N]
    m_real = mt_bf.astype(np.float64) / SV
    mpm_bf = np.stack([np.maximum(m_real, 0.0),
                       np.maximum(-m_real, 0.0)]).astype(BF)         # [2, N]

    w1 = {"q": np.asarray(inputs["wq1"], np.float32),
          "k": np.asarray(inputs["wk1"], np.float32)}
    w2 = {"q": np.asarray(inputs["wq2"], np.float32),
          "k": np.asarray(inputs["wk2"], np.float32)}
    b1 = {"q": np.asarray(inputs["bq1"], np.float32),
          "k": np.asarray(inputs["bk1"], np.float32)}
    b2 = {"q": np.asarray(inputs["bq2"], np.float32),
          "k": np.asarray(inputs["bk2"], np.float32)}

    w1_bf, w28_, wbp_, wbs_, gg_, b1_, b2_ = {}, {}, {}, {}, {}, {}, {}
    for m in "qk":
        w1_bf[m] = w1[m].astype(BF)
        w28_[m] = np.ascontiguousarray(w2[m] * SW2).astype(F8)
        wbar = w1[m].astype(np.float64).mean(axis=0)                 # [H]
        wb_bf = (wbar * SD).astype(BF)                               # wbar' bf16
        wbp_[m] = wb_bf.reshape(1, H)
        wreal = wb_bf.astype(np.float64) / SD
        # wbs = wbar * SR / SV, exact scaling of the bf16 wbar' values
        wbs_[m] = np.ascontiguousarray(
            (wb_bf.astype(np.float32) * np.float32(C1))
            .reshape(HB, P).T).astype(np.float32)                    # [P, HB]
        gp = np.maximum(wreal, 0.0) @ w2[m].astype(np.float64)
        gm = np.maximum(-wreal, 0.0) @ w2[m].astype(np.float64)
        gg_[m] = (np.stack([gp, gm]) * (SR * SW2)).astype(BF)        # [2, D]
        b1_[m] = np.ascontiguousarray(
            (b1[m] * SR).astype(np.float32).reshape(HB, P).T)        # [P, HB]
        b2_[m] = (b2[m] * FOUT).astype(np.float32).reshape(1, D)

    diag = 1.0 - 1.0 / np.sqrt(np.float64(D))
    cmask = 1.0 / (SQK * SQK)
    in_maps = []
    for c in range(NCORES):
        msk = np.full((P, D), cmask, np.float64)
        msk[np.arange(P), c * P + np.arange(P)] = diag * cmask
        im = {
            "q": np.ascontiguousarray(q_f8[c * NS:(c + 1) * NS]),
            "k": np.ascontiguousarray(k_f8[c * NS:(c + 1) * NS]),
            "vt": np.ascontiguousarray(vt_f8[:, c * NS:(c + 1) * NS]),
            "mt": np.ascontiguousarray(mt_bf[c * NS:(c + 1) * NS]).reshape(1, NS),
            "mpm": np.ascontiguousarray(mpm_bf[:, c * NS:(c + 1) * NS]),
            "mask": msk.astype(BF),
        }
        for m in "qk":
            im[f"w1s_{m}"] = np.ascontiguousarray(w1_bf[m][:, c * HS:(c + 1) * HS])
            im[f"w28_{m}"] = w28_[m]
            im[f"wbp_{m}"] = wbp_[m]
            im[f"wbl_{m}"] = np.ascontiguousarray(
                wbp_[m][:, c * HS:(c + 1) * HS])
            im[f"wbs_{m}"] = wbs_[m]
            im[f"gg_{m}"] = gg_[m]
            im[f"b1t_{m}"] = b1_[m]
            im[f"b2r_{m}"] = b2_[m]
        in_maps.append(im)
    return in_maps


def _gather(results):
    dom_q = np.concatenate([results[c]["dom_q"] for c in range(NCORES)], axis=0)
    dom_k = np.concatenate([results[c]["dom_k"] for c in range(NCORES)], axis=0)
    inv = np.float32(1.0 / FOUT)
    return dom_q.astype(np.float32) * inv, dom_k.astype(np.float32) * inv


def _run(inputs, **kw):
    from concourse import bass_utils
    nc = _get_nc()
    in_maps = _make_in_maps(inputs)
    return bass_utils.run_bass_kernel_spmd(
        nc, in_maps, core_ids=list(range(NCORES)), **kw
    )


def kernel(**inputs):
    res = _run(inputs)
    return _gather(res.results)
